# revision 36
# baseline (speedup 1.0000x reference)
"""GQA attention (RoPE, causal, per-head q-scale) on 8 TRN2 NeuronCores.

Sharding: 2-way data-parallel over batch x 4-way tensor-parallel over heads.
Core c handles batch b=c//4 and head group g=c%4 (8 q heads, 2 kv heads).
Each core computes qkv-proj -> rope -> causal attention -> partial o_proj
(over its heads' columns of Wo); the host sums the 4 bf16 partials per batch.

All scalar factors (rope_mscale, sm_scale, per_head_scale) are folded into
the Wq/Wk rows on the host.

Engine split (PE is the bottleneck; everything else is kept off its path):
  PE      all matmuls only: qkv, scores, PV, o_proj (no mask matmuls)
  Scalar  exp activations only (per (chunk, head) so PSUM score tiles
          recycle at single-bank granularity)
  Vector  rope muls/adds in all-bf16 (2x DVE mode), causal-mask multiply
          on the diag 128x128 of each band chunk (post-exp zeroing),
          softmax normalize (reciprocal straight from PSUM)
  Pool    PSUM->SBUF staging copies (rope qs, V->vaug, o_proj), output DMA

Causal masking: fully-masked column blocks are skipped (matmul widths
trimmed to the causal extent); the diagonal 128x128 triangle is zeroed in
the bf16 exp tile by a 0/1 upper-tri multiply (exact zeros, so the Vaug
ones-columns still produce exact softmax denominators).

Layouts on device (partition, free):
  xt      [hid, s]        hidden^T, streamed in 512-col chunks
  wqkv    [hid, 768]      [Wq(8 heads, scaled) | Wk(2 kv, scaled) | Wv].T
  kt_a/b  [64, s]         per-kv-head K^T at partition base 0 (no per-head
                          duplication: both q heads of a pair contract
                          against the same 64 partitions)
  qrope   [64, 2*sqw]     head A cols 0:512, head B cols 512:1024
  scores^T[sk, sq]        per (head, sk-chunk 128, sq-chunk 512), 1 PSUM bank
  exp^T   [sk, sq]        SBUF bf16 per (chunk, head), fed as matmul rhs
  Vaug    [sk, 128]       V rows (0:64) + 64 ones cols; PV matmul output
                          rows 64:128 then hold the softmax denominators
  out^T   [2d, sq]        PSUM accumulator per (head, sq-chunk)
  attn^T  [2 heads, s]    normalized bf16, lhsT for o_proj
  out     [s, hid]        partial o_proj, bf16, one [128, 2048] DMA per
                          128-row block

Emission is software-pipelined at ~1us unit granularity: attention sk
chunks are interleaved (Bresenham) with quarter-chunks of QKV(j+1) and
o_proj(j-1) blocks so the in-order PE queue always holds work that does
not depend on the in-flight exp chain.
"""

import sys, os

for _p in ("/opt/trn_rl_repo", "/root/.axon_site/_ro/trn_rl_repo"):
    if os.path.isdir(_p) and _p not in sys.path:
        sys.path.insert(0, _p)

import numpy as np

import concourse.bass as bass
import concourse.mybir as mybir
import concourse.tile as tile
from concourse import bacc
from concourse.bass_utils import run_bass_kernel_spmd

F32 = mybir.dt.float32
BF16 = mybir.dt.bfloat16
AF = mybir.ActivationFunctionType

B, S, HID = 2, 2048, 2048
H, K, D = 32, 8, 64
G = H // K
ROPE_MSCALE = 1.2
SM_SCALE = 1.0 / (D ** 0.5)

NH = 8           # q heads per core
NKV = 2          # kv heads per core
NPAIR = 4        # q head pairs per core
QO = NH * D      # 512 q rows
NK = HID // 128  # 16 contraction chunks
SQW = 512        # sq / xt chunk width
NJ = S // SQW    # 4 chunks
NSK = S // 128   # 16 sk chunks
LAG = 3          # PV chunks lag behind scores/exp emission

_CACHED = {}
DEBUG = False


def _build():
    if "nc" in _CACHED:
        return _CACHED["nc"]

    nc = bacc.Bacc(None)

    xt_d = nc.declare_dram_parameter("xt", [HID, S], BF16, isOutput=False)
    wqkv_d = nc.declare_dram_parameter("wqkv", [HID, 768], BF16, isOutput=False)
    wo_d = nc.declare_dram_parameter("wo", [QO, HID], BF16, isOutput=False)
    cost_d = nc.declare_dram_parameter("cost", [128, S], BF16, isOutput=False)
    sints_d = nc.declare_dram_parameter("sints", [128, S], BF16, isOutput=False)
    constsb_d = nc.declare_dram_parameter("constsb", [128, 264], BF16, isOutput=False)
    out_d = nc.declare_dram_parameter("out", [S, HID], BF16, isOutput=True)
    if DEBUG:
        dbg_kta_d = nc.declare_dram_parameter("dbg_kta", [64, S], BF16, isOutput=True)
        dbg_qr_d = nc.declare_dram_parameter("dbg_qr", [64, 2 * SQW], BF16, isOutput=True)
        dbg_et_d = nc.declare_dram_parameter("dbg_et", [128, SQW], BF16, isOutput=True)
        dbg_den_d = nc.declare_dram_parameter("dbg_den", [64, SQW], F32, isOutput=True)
        dbg_at_d = nc.declare_dram_parameter("dbg_at", [128, SQW], BF16, isOutput=True)

    xt_r = xt_d.rearrange("(kc p) s -> p kc s", p=128)
    wqkv_r = wqkv_d.rearrange("(k p) o -> p k o", p=128)
    wo_r = wo_d.rearrange("(m p) h -> p m h", p=128)

    with tile.TileContext(nc) as tc:
        with (
            tc.tile_pool(name="consts", bufs=1) as consts_pool,
            tc.tile_pool(name="ktv", bufs=1) as ktv_pool,
            tc.tile_pool(name="qrope", bufs=10) as qrope_pool,
            tc.tile_pool(name="expt", bufs=6) as expt_pool,
            tc.tile_pool(name="attnt", bufs=12) as attnt_pool,
            tc.tile_pool(name="inv", bufs=4) as inv_pool,
            tc.tile_pool(name="wo", bufs=1) as wo_pool,
            tc.tile_pool(name="ost", bufs=3) as ost_pool,
            tc.tile_pool(name="wq", bufs=1) as wq_pool,
            tc.tile_pool(name="xt", bufs=3) as xt_pool,
            tc.tile_pool(name="cs", bufs=1) as cs_pool,
            tc.tile_pool(name="rtmp", bufs=3) as rtmp_pool,
            tc.tile_pool(name="psc", bufs=2, space="PSUM") as psc_pool,
            tc.tile_pool(name="pout2", bufs=2, space="PSUM") as pout2_pool,
            tc.tile_pool(name="pqkv", bufs=2, space="PSUM") as pqkv_pool,
        ):
            # ---------------- initial DMA issue (priority order) ----------
            xt_tiles = {}

            def load_xt(j):
                t = xt_pool.tile([128, NK, SQW], BF16, tag="xt", name="xtt")
                for h in range(2):
                    nc.sync.dma_start(
                        out=t[:, h * 8:(h + 1) * 8, :],
                        in_=xt_r[:, h * 8:(h + 1) * 8, j * SQW:(j + 1) * SQW],
                    )
                xt_tiles[j] = t

            costd = cs_pool.tile([128, S], BF16, tag="cost", name="costd")
            sints = cs_pool.tile([128, S], BF16, tag="sints", name="sints")

            def load_cs(j):
                c0, c1 = j * SQW, (j + 1) * SQW
                nc.sync.dma_start(out=costd[:, c0:c1], in_=cost_d[:, c0:c1])
                nc.sync.dma_start(out=sints[:, c0:c1], in_=sints_d[:, c0:c1])

            # HAM warm-up: dummy matmuls with no DMA deps keep the PE busy
            # during the initial loads so the clock is ramped when real
            # work arrives.
            dummy = consts_pool.tile([128, 512], BF16, tag="dummy", name="dummy")
            nc.vector.memset(dummy, 1.0)
            pwarm = pout2_pool.tile([128, 512], F32, tag="p2", name="pwarm")
            for _ in range(24):
                nc.tensor.matmul(
                    pwarm, dummy[:, 0:128], dummy, start=True, stop=True,
                    skip_group_check=True,
                )

            # Priority-gated loads: DMA engines round-robin packets across all
            # in-flight transfers (fair share), so concurrent non-critical
            # loads starve the critical first-phase weights.  One-element
            # "poke" copies into the later groups' destinations create data
            # deps that hold those DMAs back until the prior group lands.
            wqt = wq_pool.tile([128, NK, 768], BF16, name="wqt")
            nc.sync.dma_start(out=wqt[:, 0:8, :], in_=wqkv_r[:, 0:8, :])
            load_xt(0)
            load_cs(0)
            constsb = consts_pool.tile([128, 264], BF16, name="constsb")
            nc.sync.dma_start(out=constsb, in_=constsb_d[:, :])
            nc.sync.dma_start(out=wqt[:, 8:16, :], in_=wqkv_r[:, 8:16, :])
            def gate(dst_pokes, src):
                """1-element copies into each later-group DMA's dst region:
                the DMAs WAR-wait the pokes, the pokes RAW-wait `src`."""
                for p in dst_pokes:
                    nc.gpsimd.tensor_copy(p, src)

            # group 2 (xt1, cs1): gated on group-1 completion
            xt1 = xt_pool.tile([128, NK, SQW], BF16, tag="xt", name="xtt")
            gate([xt1[0:1, 0, 0:1], xt1[0:1, 8, 0:1],
                  costd[0:1, SQW:SQW + 1], sints[0:1, SQW:SQW + 1]],
                 wqt[0:1, 15, 767:768])
            for h in range(2):
                nc.sync.dma_start(
                    out=xt1[:, h * 8:(h + 1) * 8, :],
                    in_=xt_r[:, h * 8:(h + 1) * 8, SQW:2 * SQW],
                )
            xt_tiles[1] = xt1
            load_cs(1)
            # group 3 (wot, xt2, cs2, cs3): gated on group-2 completion
            wot = wo_pool.tile([128, NPAIR, HID], BF16, name="wot")
            xt2 = xt_pool.tile([128, NK, SQW], BF16, tag="xt", name="xtt")
            gate([wot[0:1, 0, 0:1], wot[0:1, 2, 0:1],
                  xt2[0:1, 0, 0:1], xt2[0:1, 8, 0:1],
                  costd[0:1, 2 * SQW:2 * SQW + 1], sints[0:1, 2 * SQW:2 * SQW + 1],
                  costd[0:1, 3 * SQW:3 * SQW + 1], sints[0:1, 3 * SQW:3 * SQW + 1]],
                 xt1[0:1, 15, 511:512])
            for h in range(2):
                nc.sync.dma_start(
                    out=wot[:, h * 2:(h + 1) * 2, :], in_=wo_r[:, h * 2:(h + 1) * 2, :]
                )
                nc.sync.dma_start(
                    out=xt2[:, h * 8:(h + 1) * 8, :],
                    in_=xt_r[:, h * 8:(h + 1) * 8, 2 * SQW:3 * SQW],
                )
            xt_tiles[2] = xt2
            load_cs(2)
            load_cs(3)

            tri01b = constsb[:, 0:128]      # upper-tri (incl diag) 0/1
            ones_colb = constsb[:, 128:129]
            id128 = constsb[:, 136:264]     # identity (PE transpose rhs)

            kt_a = ktv_pool.tile([64, S], BF16, tag="kta", name="kta")
            kt_b = ktv_pool.tile([64, S], BF16, tag="ktb", name="ktb")
            # Vaug: cols 0:64 = V, cols 64:128 = 1.0 (sums -> rows 64:128 of
            # the PV output = softmax denominators, pre-broadcast)
            vaug = [
                ktv_pool.tile([128, NSK, 128], BF16, tag=f"vaug{i}", name=f"vaug{i}")
                for i in range(NKV)
            ]
            for i in range(NKV):
                nc.vector.tensor_copy(
                    vaug[i][:, :, 64:128],
                    ones_colb[:, None, :].broadcast_to([128, NSK, 64]),
                )

            qrope = {}   # (m, j) -> tile [64, 2*SQW] bf16
            attnt = {}   # (m, j) -> tile [128, SQW] bf16

            # ---------------- emission building blocks --------------------
            def rope_emit(pq, j, m):
                """q' = q*cos + swap32(q)*sin_signed, all bf16 on DVE.
                pq is the projected [128, SQW] psum chunk (2 heads), staged
                to SBUF bf16 (freeing the PSUM bank); the four swap-
                multiplies and the final adds run as bf16 SBUF ops (2x DVE
                mode).  Head outputs land at partition base 0.  j=0 ropes
                run before any exp exists, so the scalar engine stages them
                (the vector queue would hold the PSUM bank hostage)."""
                c0, c1 = j * SQW, (j + 1) * SQW
                qs = rtmp_pool.tile([128, SQW], BF16, tag="qs", name="qs")
                if j == 0:
                    nc.scalar.copy(qs, pq)
                else:
                    nc.vector.tensor_copy(qs, pq)
                # sints is partition-permuted on the host (swap32 within each
                # 64-block) so both SBUF inputs of each mul share a base
                # partition (NCC_IBIR297); only the output is shifted.
                t2 = rtmp_pool.tile([128, SQW], BF16, tag="t2", name="t2")
                for base in (0, 64):
                    nc.vector.tensor_mul(
                        t2[base:base + 32, :], qs[base + 32:base + 64, :],
                        sints[base + 32:base + 64, c0:c1],
                    )
                    nc.vector.tensor_mul(
                        t2[base + 32:base + 64, :], qs[base:base + 32, :],
                        sints[base:base + 32, c0:c1],
                    )
                t4 = rtmp_pool.tile([128, SQW], BF16, tag="t4", name="t4")
                nc.vector.tensor_mul(t4, qs, costd[:, c0:c1])
                if m < NPAIR:
                    qr = qrope_pool.tile([64, 2 * SQW], BF16, tag="qr", name="qr")
                    qrope[(m, j)] = qr
                    nc.vector.tensor_add(qr[:, 0:SQW], t2[0:64, :], t4[0:64, :])
                    nc.vector.tensor_add(qr[:, SQW:2 * SQW], t2[64:128, :], t4[64:128, :])
                    if DEBUG and (m, j) == (0, 0):
                        nc.gpsimd.dma_start(out=dbg_qr_d[:, :], in_=qr)
                else:
                    nc.vector.tensor_add(kt_a[:, c0:c1], t2[0:64, :], t4[0:64, :])
                    nc.vector.tensor_add(kt_b[:, c0:c1], t2[64:128, :], t4[64:128, :])
                    if DEBUG:
                        nc.gpsimd.dma_start(out=dbg_kta_d[:, c0:c1], in_=kt_a[:, c0:c1])

            def qkv_units(j, m):
                """Generator: 4 units. m 0..3 -> q pair chunk, 4 -> K, 5 -> V."""
                xt_t = xt_tiles[j]
                if m != 5:
                    pq = pqkv_pool.tile([128, SQW], F32, tag="qkv", name="pqkv")
                    for q in range(4):
                        for k in range(4 * q, 4 * q + 4):
                            nc.tensor.matmul(
                                pq, wqt[:, k, m * 128:(m + 1) * 128], xt_t[:, k, :],
                                start=(k == 0), stop=(k == NK - 1),
                            )
                        if q == 3:
                            rope_emit(pq, j, m)
                        yield
                else:
                    # V projected wide in [d, s] orientation (16 matmuls of
                    # N=512 instead of 64 of N=128), then PE-transposed back
                    # to [s, d] for the vaug layout
                    pqv = pqkv_pool.tile([128, SQW], F32, tag="qkv", name="pqv")
                    vstage = rtmp_pool.tile([128, SQW], BF16, tag="vst", name="vst")
                    for q in range(4):
                        for k in range(4 * q, 4 * q + 4):
                            nc.tensor.matmul(
                                pqv, wqt[:, k, 640:768], xt_t[:, k, :],
                                start=(k == 0), stop=(k == NK - 1),
                            )
                        if q == 3:
                            if j == 0:
                                nc.scalar.copy(vstage, pqv)
                            else:
                                nc.vector.tensor_copy(vstage, pqv)
                        yield
                    for h2 in range(4):
                        sk = j * 4 + h2
                        pvt = pqkv_pool.tile([128, 128], BF16, tag="qkv", name="pvt")
                        nc.tensor.transpose(
                            pvt, vstage[:, h2 * 128:(h2 + 1) * 128], id128
                        )
                        for i in range(NKV):
                            nc.vector.tensor_copy(
                                vaug[i][:, sk, 0:64], pvt[:, i * 64:(i + 1) * 64]
                            )
                        if h2 % 2 == 1:
                            yield

            _done_pairs = set()

            def attention_pair_units(j, m):
                """Generator: one yield per sk chunk.  Emits the two heads'
                scores matmuls + exps (+ diag mask) for chunk sk, and the PV
                matmuls of chunk sk-LAG; drains and normalizes at the end."""
                if (j, m) in _done_pairs:
                    return
                _done_pairs.add((j, m))
                nsk = 4 * (j + 1)
                kt = kt_a if m < 2 else kt_b
                va = vaug[m // 2]
                qr = qrope.pop((m, j))
                p2 = [
                    pout2_pool.tile([128, SQW], F32, tag="p2", name="p2")
                    for _ in range(2)
                ]
                pend = []

                def pv_emit(et, psk):
                    poff = (psk - 4 * j) * 128 if psk >= 4 * j else 0
                    for hi in (0, 1):
                        nc.tensor.matmul(
                            p2[hi][:, poff:SQW], va[:, psk, :],
                            et[:, hi, poff:SQW],
                            start=(psk == 0), stop=(psk == nsk - 1),
                            skip_group_check=True,
                        )

                for sk in range(nsk):
                    band = sk >= 4 * j
                    off = (sk - 4 * j) * 128 if band else 0
                    # both heads' scores side by side in one 2-bank tile so a
                    # single wide exp covers them (halves scalar sync points)
                    p1 = psc_pool.tile([128, 2, SQW], F32, tag="sc", name="sc")
                    for hi in (0, 1):
                        nc.tensor.matmul(
                            p1[:, hi, off:SQW],
                            kt[:, sk * 128:(sk + 1) * 128],
                            qr[:, hi * SQW + off:(hi + 1) * SQW],
                            start=True, stop=True, skip_group_check=True,
                        )
                    et = expt_pool.tile([128, 2, SQW], BF16, tag="et", name="et")
                    nc.scalar.activation(et[:, :, off:SQW], p1[:, :, off:SQW], AF.Exp)
                    if band:
                        # zero the causally-invalid lower triangle of the
                        # diagonal 128x128 (exact zeros in bf16)
                        for hi in (0, 1):
                            nc.vector.tensor_mul(
                                et[:, hi, off:off + 128],
                                et[:, hi, off:off + 128], tri01b
                            )
                    if DEBUG and (j, m, sk) == (0, 0, 0):
                        nc.gpsimd.dma_start(out=dbg_et_d[:, :], in_=et[:, 0, :])
                    pend.append((et, sk))
                    if len(pend) > LAG:
                        pv_emit(*pend.pop(0))
                    yield
                while pend:
                    pv_emit(*pend.pop(0))
                # normalize: attnT = out^T * (1/sums); sums sit pre-broadcast
                # in psum rows 64:128
                at = attnt_pool.tile([128, SQW], BF16, tag="at", name="at")
                if DEBUG and (j, m) == (0, 0):
                    dent = inv_pool.tile([64, SQW], F32, tag="dent", name="dent")
                    nc.vector.tensor_copy(dent, p2[0][64:128, :])
                    nc.gpsimd.dma_start(out=dbg_den_d[:, :], in_=dent)
                # stage both heads' denominators to SBUF before the custom-DVE
                # reciprocal: reciprocal_approx_fast reading PSUM at a shifted
                # base silently corrupts on hardware (sim-only correct)
                sums = inv_pool.tile([128, SQW], F32, tag="sums", name="sums")
                nc.vector.tensor_copy(sums[0:64, :], p2[0][64:128, :])
                nc.vector.tensor_copy(sums[64:128, :], p2[1][64:128, :])
                invb = inv_pool.tile([128, SQW], F32, tag="invb", name="invb")
                nc.vector.reciprocal_approx_fast(out=invb, in_=sums)
                for hi in (0, 1):
                    nc.vector.tensor_mul(
                        at[hi * 64:(hi + 1) * 64, :], p2[hi][0:64, :],
                        invb[hi * 64:(hi + 1) * 64, :],
                    )
                attnt[(m, j)] = at
                if DEBUG and (j, m) == (0, 0):
                    nc.gpsimd.dma_start(out=dbg_at_d[:, :], in_=at)

            def oproj_units(j, sc):
                """Generator: 4 units (one per 512-col block of Wo); each
                block's DMA starts as soon as its copy lands."""
                r0 = j * SQW + sc * 128
                for hc in range(4):
                    # the tail (last j) has no live attention pairs, so the
                    # pout2 banks are free — alternate to double-buffer deeper
                    pool = pout2_pool if (j == NJ - 1 and hc % 2) else pqkv_pool
                    po = pool.tile([128, 512], F32,
                                   tag="p2" if pool is pout2_pool else "qkv",
                                   name="po")
                    for mm in range(NPAIR):
                        nc.tensor.matmul(
                            po, attnt[(mm, j)][:, sc * 128:(sc + 1) * 128],
                            wot[:, mm, hc * 512:(hc + 1) * 512],
                            start=(mm == 0), stop=(mm == NPAIR - 1),
                        )
                    ot = ost_pool.tile([128, 512], BF16, tag="ot", name="ot")
                    if j == NJ - 1:
                        # tail: no exps remain, scalar is idle and the vector
                        # queue is backed up with the last pair's normalize
                        nc.scalar.copy(ot, po)
                    else:
                        nc.vector.tensor_copy(ot, po)
                    # hwdge (sync) spreads the write over all 16 DMA engines;
                    # the swdge path serializes on 2-3 and drags out the tail
                    nc.sync.dma_start(
                        out=out_d[r0:r0 + 128, hc * 512:(hc + 1) * 512], in_=ot
                    )
                    if hc == 3 and sc == 3:
                        for mm in range(NPAIR):
                            attnt.pop((mm, j))
                    yield

            def drive(pairs, fillers):
                """Run pair generators in order, spreading filler units
                evenly (Bresenham) across the total chunk count."""
                from itertools import chain
                n_chunks = sum(n for _, n in pairs)
                n_fill = sum(n for _, n in fillers)
                fit = chain(*(g for g, _ in fillers))
                done_c = emitted_f = 0
                for g, n in pairs:
                    for _ in g:
                        done_c += 1
                        target = (n_fill * done_c) // max(1, n_chunks)
                        while emitted_f < target:
                            next(fit, None)
                            emitted_f += 1
                for _ in fit:
                    pass

            # ---------------- schedule -----------------------------------
            QKV_ORDER = (4, 0, 5, 1, 2, 3)   # K first, V early, q pairs after

            # j=0 head: all of QKV(0) inline — its ~21us of PE work hides the
            # rope chains, so attention never waits on the vector engine
            for m in QKV_ORDER:
                for _ in qkv_units(0, m):
                    pass
            drive([(attention_pair_units(0, 0), 4)],
                  [(qkv_units(1, 4), 4), (qkv_units(1, 0), 4)])
            drive([(attention_pair_units(0, 1), 4)],
                  [(qkv_units(1, 5), 6), (qkv_units(1, 1), 4)])
            drive([(attention_pair_units(0, 2), 4)],
                  [(qkv_units(1, 2), 4)])
            drive([(attention_pair_units(0, 3), 4)],
                  [(qkv_units(1, 3), 4)])
            xt_tiles.pop(0)

            for j in range(1, NJ):
                if j + 2 < NJ:
                    load_xt(j + 2)
                nsk = 4 * (j + 1)
                pairs = [(attention_pair_units(j, m), nsk) for m in range(NPAIR)
                         if (j, m) not in _done_pairs]
                fillers = []
                if j + 1 < NJ:
                    fillers += [(qkv_units(j + 1, m), 6 if m == 5 else 4)
                                for m in QKV_ORDER]
                fillers += [(oproj_units(j - 1, sc), 4) for sc in range(4)]
                if j == NJ - 2:
                    # pull two of the last chunk's head-pairs forward so the
                    # tail window keeps the PE fed
                    pairs += [(attention_pair_units(NJ - 1, 0), 4 * NJ),
                              (attention_pair_units(NJ - 1, 1), 4 * NJ)]
                drive(pairs, fillers)
                xt_tiles.pop(j, None)
            for sc in range(4):
                for _ in oproj_units(NJ - 1, sc):
                    pass

    nc.finalize()
    _CACHED["nc"] = nc
    return nc


def _prep_inputs(cos, sin, hidden_states, per_head_scale, Wqkv, Wo):
    """Build the 8 per-core input maps (host-side, free)."""
    import ml_dtypes
    cos = np.asarray(cos, np.float32)
    sin = np.asarray(sin, np.float32)
    hs = np.asarray(hidden_states, np.float32)
    phs = np.asarray(per_head_scale, np.float32)
    Wqkv = np.asarray(Wqkv, np.float32)
    Wo = np.asarray(Wo, np.float32)

    cost = np.ascontiguousarray(np.vstack([cos.T, cos.T])).astype(ml_dtypes.bfloat16)
    st = sin.T.copy()
    st[0:32] *= -1.0
    # swap32-permuted: the sin factor for output row r is stored at row
    # partner(r), matching the partition of the swapped q operand
    stx = np.concatenate([st[32:64], st[0:32]], axis=0)
    sints = np.ascontiguousarray(np.vstack([stx, stx])).astype(ml_dtypes.bfloat16)

    tri01 = np.zeros((128, 128), np.float32)
    for p in range(128):
        tri01[p, p:] = 1.0
    ones_pad = np.zeros((128, 8), np.float32)
    ones_pad[:, 0] = 1.0
    id128 = np.eye(128, dtype=np.float32)
    constsb = np.ascontiguousarray(
        np.concatenate([tri01, ones_pad, id128], axis=1)
    ).astype(ml_dtypes.bfloat16)

    xt_b = [np.ascontiguousarray(hs[b].T).astype(ml_dtypes.bfloat16) for b in range(B)]

    in_maps = []
    for c in range(8):
        b, g = c // 4, c % 4
        hq0 = NH * g
        wq = Wqkv[hq0 * D:(hq0 + NH) * D, :].copy()
        for h in range(NH):
            wq[h * D:(h + 1) * D] *= (
                ROPE_MSCALE * SM_SCALE * phs[b, hq0 + h]
            )
        kv0 = H * D + NKV * g * D
        wk = Wqkv[kv0:kv0 + NKV * D, :] * ROPE_MSCALE
        v0 = (H + K) * D + NKV * g * D
        wv = Wqkv[v0:v0 + NKV * D, :]
        wqkv_c = np.ascontiguousarray(np.concatenate([wq, wk, wv], axis=0).T).astype(ml_dtypes.bfloat16)
        in_maps.append({
            "xt": xt_b[b],
            "wqkv": wqkv_c,
            "wo": np.ascontiguousarray(
                Wo[:, hq0 * D:(hq0 + NH) * D].T
            ).astype(ml_dtypes.bfloat16),
            "cost": cost,
            "sints": sints,
            "constsb": constsb,
        })
    return in_maps


def kernel(cos, sin, hidden_states, per_head_scale, Wqkv, Wo, _trace=False):
    nc = _build()
    in_maps = _prep_inputs(cos, sin, hidden_states, per_head_scale, Wqkv, Wo)
    res = run_bass_kernel_spmd(nc, in_maps, core_ids=list(range(8)), trace=_trace)
    _CACHED["last_results"] = res
    out = np.stack([
        sum(res.results[b * 4 + g]["out"].astype(np.float64) for g in range(4))
        for b in range(B)
    ]).astype(np.float32)
    return out


# revision 37
# speedup vs baseline: 1.1660x; 1.1660x over previous
"""GQA attention (RoPE, causal, per-head q-scale) on 8 TRN2 NeuronCores.

Sharding: 2-way data-parallel over batch x 4-way tensor-parallel over heads.
Core c handles batch b=c//4 and head group g=c%4 (8 q heads, 2 kv heads).
Each core computes qkv-proj -> rope -> causal attention -> partial o_proj
(over its heads' columns of Wo); the host sums the 4 bf16 partials per batch.

All scalar factors (rope_mscale, sm_scale, per_head_scale) are folded into
the Wq/Wk rows on the host.

Engine split (PE is the bottleneck; everything else is kept off its path):
  PE      all matmuls only: qkv, scores, PV, o_proj (no mask matmuls)
  Scalar  exp activations only (per (chunk, head) so PSUM score tiles
          recycle at single-bank granularity)
  Vector  rope muls/adds in all-bf16 (2x DVE mode), causal-mask multiply
          on the diag 128x128 of each band chunk (post-exp zeroing),
          softmax normalize (reciprocal straight from PSUM)
  Pool    PSUM->SBUF staging copies (rope qs, V->vaug, o_proj), output DMA

Causal masking: fully-masked column blocks are skipped (matmul widths
trimmed to the causal extent); the diagonal 128x128 triangle is zeroed in
the bf16 exp tile by a 0/1 upper-tri multiply (exact zeros, so the Vaug
ones-columns still produce exact softmax denominators).

Layouts on device (partition, free):
  xt      [hid, s]        hidden^T, streamed in 512-col chunks
  wqkv    [hid, 768]      [Wq(8 heads, scaled) | Wk(2 kv, scaled) | Wv].T
  kt_a/b  [64, s]         per-kv-head K^T at partition base 0 (no per-head
                          duplication: both q heads of a pair contract
                          against the same 64 partitions)
  qrope   [64, 2*sqw]     head A cols 0:512, head B cols 512:1024
  scores^T[sk, sq]        per (head, sk-chunk 128, sq-chunk 512), 1 PSUM bank
  exp^T   [sk, sq]        SBUF bf16 per (chunk, head), fed as matmul rhs
  Vaug    [sk, 128]       V rows (0:64) + 64 ones cols; PV matmul output
                          rows 64:128 then hold the softmax denominators
  out^T   [2d, sq]        PSUM accumulator per (head, sq-chunk)
  attn^T  [2 heads, s]    normalized bf16, lhsT for o_proj
  out     [s, hid]        partial o_proj, bf16, one [128, 2048] DMA per
                          128-row block

Emission is software-pipelined at ~1us unit granularity: attention sk
chunks are interleaved (Bresenham) with quarter-chunks of QKV(j+1) and
o_proj(j-1) blocks so the in-order PE queue always holds work that does
not depend on the in-flight exp chain.
"""

import sys, os

for _p in ("/opt/trn_rl_repo", "/root/.axon_site/_ro/trn_rl_repo"):
    if os.path.isdir(_p) and _p not in sys.path:
        sys.path.insert(0, _p)

import numpy as np

import concourse.bass as bass
import concourse.mybir as mybir
import concourse.tile as tile
from concourse import bacc
from concourse.bass_utils import run_bass_kernel_spmd

F32 = mybir.dt.float32
BF16 = mybir.dt.bfloat16
AF = mybir.ActivationFunctionType

B, S, HID = 2, 2048, 2048
H, K, D = 32, 8, 64
G = H // K
ROPE_MSCALE = 1.2
SM_SCALE = 1.0 / (D ** 0.5)

NH = 8           # q heads per core
NKV = 2          # kv heads per core
NPAIR = 4        # q head pairs per core
QO = NH * D      # 512 q rows
NK = HID // 128  # 16 contraction chunks
SQW = 512        # sq / xt chunk width
NJ = S // SQW    # 4 chunks
NSK = S // 128   # 16 sk chunks
LAG = 3          # PV chunks lag behind scores/exp emission

_CACHED = {}
DEBUG = False


def _build():
    if "nc" in _CACHED:
        return _CACHED["nc"]

    nc = bacc.Bacc(None)

    xt_d = nc.declare_dram_parameter("xt", [HID, S], BF16, isOutput=False)
    wqkv_d = nc.declare_dram_parameter("wqkv", [HID, 768], BF16, isOutput=False)
    wo_d = nc.declare_dram_parameter("wo", [QO, HID], BF16, isOutput=False)
    cost_d = nc.declare_dram_parameter("cost", [128, S], BF16, isOutput=False)
    sints_d = nc.declare_dram_parameter("sints", [128, S], BF16, isOutput=False)
    constsb_d = nc.declare_dram_parameter("constsb", [128, 264], BF16, isOutput=False)
    out_d = nc.declare_dram_parameter("out", [S, HID], BF16, isOutput=True)
    if DEBUG:
        dbg_kta_d = nc.declare_dram_parameter("dbg_kta", [64, S], BF16, isOutput=True)
        dbg_qr_d = nc.declare_dram_parameter("dbg_qr", [64, 2 * SQW], BF16, isOutput=True)
        dbg_et_d = nc.declare_dram_parameter("dbg_et", [128, SQW], BF16, isOutput=True)
        dbg_den_d = nc.declare_dram_parameter("dbg_den", [64, SQW], F32, isOutput=True)
        dbg_at_d = nc.declare_dram_parameter("dbg_at", [128, SQW], BF16, isOutput=True)

    xt_r = xt_d.rearrange("(kc p) s -> p kc s", p=128)
    wqkv_r = wqkv_d.rearrange("(k p) o -> p k o", p=128)
    wo_r = wo_d.rearrange("(m p) h -> p m h", p=128)

    with tile.TileContext(nc) as tc:
        with (
            tc.tile_pool(name="consts", bufs=1) as consts_pool,
            tc.tile_pool(name="ktv", bufs=1) as ktv_pool,
            tc.tile_pool(name="qrope", bufs=10) as qrope_pool,
            tc.tile_pool(name="expt", bufs=6) as expt_pool,
            tc.tile_pool(name="attnt", bufs=12) as attnt_pool,
            tc.tile_pool(name="inv", bufs=4) as inv_pool,
            tc.tile_pool(name="wo", bufs=1) as wo_pool,
            tc.tile_pool(name="ost", bufs=3) as ost_pool,
            tc.tile_pool(name="wq", bufs=1) as wq_pool,
            tc.tile_pool(name="xt", bufs=3) as xt_pool,
            tc.tile_pool(name="cs", bufs=1) as cs_pool,
            tc.tile_pool(name="rtmp", bufs=3) as rtmp_pool,
            tc.tile_pool(name="psc", bufs=2, space="PSUM") as psc_pool,
            tc.tile_pool(name="pout2", bufs=2, space="PSUM") as pout2_pool,
            tc.tile_pool(name="pqkv", bufs=2, space="PSUM") as pqkv_pool,
        ):
            # ---------------- initial DMA issue (priority order) ----------
            xt_tiles = {}

            def load_xt(j):
                t = xt_pool.tile([128, NK, SQW], BF16, tag="xt", name="xtt")
                for h in range(2):
                    nc.sync.dma_start(
                        out=t[:, h * 8:(h + 1) * 8, :],
                        in_=xt_r[:, h * 8:(h + 1) * 8, j * SQW:(j + 1) * SQW],
                    )
                xt_tiles[j] = t

            costd = cs_pool.tile([128, S], BF16, tag="cost", name="costd")
            sints = cs_pool.tile([128, S], BF16, tag="sints", name="sints")

            def load_cs(j):
                c0, c1 = j * SQW, (j + 1) * SQW
                nc.sync.dma_start(out=costd[:, c0:c1], in_=cost_d[:, c0:c1])
                nc.sync.dma_start(out=sints[:, c0:c1], in_=sints_d[:, c0:c1])

            # HAM warm-up: dummy matmuls with no DMA deps keep the PE busy
            # during the initial loads so the clock is ramped when real
            # work arrives.
            dummy = consts_pool.tile([128, 512], BF16, tag="dummy", name="dummy")
            nc.vector.memset(dummy, 1.0)
            pwarm = pout2_pool.tile([128, 512], F32, tag="p2", name="pwarm")
            for _ in range(24):
                nc.tensor.matmul(
                    pwarm, dummy[:, 0:128], dummy, start=True, stop=True,
                    skip_group_check=True,
                )

            # Priority-gated loads: DMA engines round-robin packets across all
            # in-flight transfers (fair share), so concurrent non-critical
            # loads starve the critical first-phase weights.  One-element
            # "poke" copies into the later groups' destinations create data
            # deps that hold those DMAs back until the prior group lands.
            wqt = wq_pool.tile([128, NK, 768], BF16, name="wqt")
            nc.sync.dma_start(out=wqt[:, 0:8, :], in_=wqkv_r[:, 0:8, :])
            load_xt(0)
            load_cs(0)
            constsb = consts_pool.tile([128, 264], BF16, name="constsb")
            nc.sync.dma_start(out=constsb, in_=constsb_d[:, :])
            nc.sync.dma_start(out=wqt[:, 8:16, :], in_=wqkv_r[:, 8:16, :])
            def gate(dst_pokes, src):
                """1-element copies into each later-group DMA's dst region:
                the DMAs WAR-wait the pokes, the pokes RAW-wait `src`."""
                for p in dst_pokes:
                    nc.gpsimd.tensor_copy(p, src)

            # group 2 (xt1, cs1): gated on group-1 completion
            xt1 = xt_pool.tile([128, NK, SQW], BF16, tag="xt", name="xtt")
            gate([xt1[0:1, 0, 0:1], xt1[0:1, 8, 0:1],
                  costd[0:1, SQW:SQW + 1], sints[0:1, SQW:SQW + 1]],
                 wqt[0:1, 15, 767:768])
            for h in range(2):
                nc.sync.dma_start(
                    out=xt1[:, h * 8:(h + 1) * 8, :],
                    in_=xt_r[:, h * 8:(h + 1) * 8, SQW:2 * SQW],
                )
            xt_tiles[1] = xt1
            load_cs(1)
            # group 3 (wot, xt2, cs2, cs3): gated on group-2 completion
            wot = wo_pool.tile([128, NPAIR, HID], BF16, name="wot")
            xt2 = xt_pool.tile([128, NK, SQW], BF16, tag="xt", name="xtt")
            gate([wot[0:1, 0, 0:1], wot[0:1, 2, 0:1],
                  xt2[0:1, 0, 0:1], xt2[0:1, 8, 0:1],
                  costd[0:1, 2 * SQW:2 * SQW + 1], sints[0:1, 2 * SQW:2 * SQW + 1],
                  costd[0:1, 3 * SQW:3 * SQW + 1], sints[0:1, 3 * SQW:3 * SQW + 1]],
                 xt1[0:1, 15, 511:512])
            for h in range(2):
                nc.sync.dma_start(
                    out=wot[:, h * 2:(h + 1) * 2, :], in_=wo_r[:, h * 2:(h + 1) * 2, :]
                )
                nc.sync.dma_start(
                    out=xt2[:, h * 8:(h + 1) * 8, :],
                    in_=xt_r[:, h * 8:(h + 1) * 8, 2 * SQW:3 * SQW],
                )
            xt_tiles[2] = xt2
            load_cs(2)
            load_cs(3)

            tri01b = constsb[:, 0:128]      # upper-tri (incl diag) 0/1
            ones_colb = constsb[:, 128:129]
            id128 = constsb[:, 136:264]     # identity (PE transpose rhs)

            kt_a = ktv_pool.tile([64, S], BF16, tag="kta", name="kta")
            kt_b = ktv_pool.tile([64, S], BF16, tag="ktb", name="ktb")
            # Vaug: cols 0:64 = V, cols 64:128 = 1.0 (sums -> rows 64:128 of
            # the PV output = softmax denominators, pre-broadcast)
            vaug = [
                ktv_pool.tile([128, NSK, 128], BF16, tag=f"vaug{i}", name=f"vaug{i}")
                for i in range(NKV)
            ]
            for i in range(NKV):
                nc.vector.tensor_copy(
                    vaug[i][:, :, 64:128],
                    ones_colb[:, None, :].broadcast_to([128, NSK, 64]),
                )

            qrope = {}   # (m, j) -> tile [64, 2*SQW] bf16
            attnt = {}   # (m, j) -> tile [128, SQW] bf16

            # ---------------- emission building blocks --------------------
            def rope_emit(pq, j, m):
                """q' = q*cos + swap32(q)*sin_signed, all bf16 on DVE.
                pq is the projected [128, SQW] psum chunk (2 heads), staged
                to SBUF bf16 (freeing the PSUM bank); the four swap-
                multiplies and the final adds run as bf16 SBUF ops (2x DVE
                mode).  Head outputs land at partition base 0.  j=0 ropes
                run before any exp exists, so the scalar engine stages them
                (the vector queue would hold the PSUM bank hostage)."""
                c0, c1 = j * SQW, (j + 1) * SQW
                qs = rtmp_pool.tile([128, SQW], BF16, tag="qs", name="qs")
                if j == 0:
                    nc.scalar.copy(qs, pq)
                else:
                    nc.vector.tensor_copy(qs, pq)
                # sints is partition-permuted on the host (swap32 within each
                # 64-block) so both SBUF inputs of each mul share a base
                # partition (NCC_IBIR297); only the output is shifted.
                t2 = rtmp_pool.tile([128, SQW], BF16, tag="t2", name="t2")
                for base in (0, 64):
                    nc.vector.tensor_mul(
                        t2[base:base + 32, :], qs[base + 32:base + 64, :],
                        sints[base + 32:base + 64, c0:c1],
                    )
                    nc.vector.tensor_mul(
                        t2[base + 32:base + 64, :], qs[base:base + 32, :],
                        sints[base:base + 32, c0:c1],
                    )
                t4 = rtmp_pool.tile([128, SQW], BF16, tag="t4", name="t4")
                nc.vector.tensor_mul(t4, qs, costd[:, c0:c1])
                if m < NPAIR:
                    qr = qrope_pool.tile([64, 2 * SQW], BF16, tag="qr", name="qr")
                    qrope[(m, j)] = qr
                    nc.vector.tensor_add(qr[:, 0:SQW], t2[0:64, :], t4[0:64, :])
                    nc.vector.tensor_add(qr[:, SQW:2 * SQW], t2[64:128, :], t4[64:128, :])
                    if DEBUG and (m, j) == (0, 0):
                        nc.gpsimd.dma_start(out=dbg_qr_d[:, :], in_=qr)
                else:
                    nc.vector.tensor_add(kt_a[:, c0:c1], t2[0:64, :], t4[0:64, :])
                    nc.vector.tensor_add(kt_b[:, c0:c1], t2[64:128, :], t4[64:128, :])
                    if DEBUG:
                        nc.gpsimd.dma_start(out=dbg_kta_d[:, c0:c1], in_=kt_a[:, c0:c1])

            def qkv_units(j, m):
                """Generator: 4 units. m 0..3 -> q pair chunk, 4 -> K, 5 -> V."""
                xt_t = xt_tiles[j]
                if m != 5:
                    pq = pqkv_pool.tile([128, SQW], F32, tag="qkv", name="pqkv")
                    for q in range(4):
                        for k in range(4 * q, 4 * q + 4):
                            nc.tensor.matmul(
                                pq, wqt[:, k, m * 128:(m + 1) * 128], xt_t[:, k, :],
                                start=(k == 0), stop=(k == NK - 1),
                            )
                        if q == 3:
                            rope_emit(pq, j, m)
                        yield
                else:
                    # V projected wide in [d, s] orientation (16 matmuls of
                    # N=512 instead of 64 of N=128), then PE-transposed back
                    # to [s, d] for the vaug layout
                    pqv = pqkv_pool.tile([128, SQW], F32, tag="qkv", name="pqv")
                    vstage = rtmp_pool.tile([128, SQW], BF16, tag="vst", name="vst")
                    for q in range(4):
                        for k in range(4 * q, 4 * q + 4):
                            nc.tensor.matmul(
                                pqv, wqt[:, k, 640:768], xt_t[:, k, :],
                                start=(k == 0), stop=(k == NK - 1),
                            )
                        if q == 3:
                            if j == 0:
                                nc.scalar.copy(vstage, pqv)
                            else:
                                nc.vector.tensor_copy(vstage, pqv)
                        yield
                    for h2 in range(4):
                        sk = j * 4 + h2
                        pvt = pqkv_pool.tile([128, 128], BF16, tag="qkv", name="pvt")
                        nc.tensor.transpose(
                            pvt, vstage[:, h2 * 128:(h2 + 1) * 128], id128
                        )
                        for i in range(NKV):
                            nc.vector.tensor_copy(
                                vaug[i][:, sk, 0:64], pvt[:, i * 64:(i + 1) * 64]
                            )
                        if h2 % 2 == 1:
                            yield

            _done_pairs = set()

            def attention_pair_units(j, m):
                """Generator: one yield per sk chunk.  Emits the two heads'
                scores matmuls + exps (+ diag mask) for chunk sk, and the PV
                matmuls of chunk sk-LAG; drains and normalizes at the end."""
                if (j, m) in _done_pairs:
                    return
                _done_pairs.add((j, m))
                nsk = 4 * (j + 1)
                kt = kt_a if m < 2 else kt_b
                va = vaug[m // 2]
                qr = qrope.pop((m, j))
                p2 = [
                    pout2_pool.tile([128, SQW], F32, tag="p2", name="p2")
                    for _ in range(2)
                ]
                pend = []

                def pv_emit(et, psk):
                    poff = (psk - 4 * j) * 128 if psk >= 4 * j else 0
                    for hi in (0, 1):
                        nc.tensor.matmul(
                            p2[hi][:, poff:SQW], va[:, psk, :],
                            et[:, hi, poff:SQW],
                            start=(psk == 0), stop=(psk == nsk - 1),
                            skip_group_check=True,
                        )

                for sk in range(nsk):
                    band = sk >= 4 * j
                    off = (sk - 4 * j) * 128 if band else 0
                    # both heads' scores side by side in one 2-bank tile so a
                    # single wide exp covers them (halves scalar sync points)
                    p1 = psc_pool.tile([128, 2, SQW], F32, tag="sc", name="sc")
                    for hi in (0, 1):
                        nc.tensor.matmul(
                            p1[:, hi, off:SQW],
                            kt[:, sk * 128:(sk + 1) * 128],
                            qr[:, hi * SQW + off:(hi + 1) * SQW],
                            start=True, stop=True, skip_group_check=True,
                        )
                    et = expt_pool.tile([128, 2, SQW], BF16, tag="et", name="et")
                    nc.scalar.activation(et[:, :, off:SQW], p1[:, :, off:SQW], AF.Exp)
                    if band:
                        # zero the causally-invalid lower triangle of the
                        # diagonal 128x128 (exact zeros in bf16)
                        for hi in (0, 1):
                            nc.vector.tensor_mul(
                                et[:, hi, off:off + 128],
                                et[:, hi, off:off + 128], tri01b
                            )
                    if DEBUG and (j, m, sk) == (0, 0, 0):
                        nc.gpsimd.dma_start(out=dbg_et_d[:, :], in_=et[:, 0, :])
                    pend.append((et, sk))
                    if len(pend) > LAG:
                        pv_emit(*pend.pop(0))
                    yield
                while pend:
                    pv_emit(*pend.pop(0))
                # normalize: attnT = out^T * (1/sums); sums sit pre-broadcast
                # in psum rows 64:128
                at = attnt_pool.tile([128, SQW], BF16, tag="at", name="at")
                if DEBUG and (j, m) == (0, 0):
                    dent = inv_pool.tile([64, SQW], F32, tag="dent", name="dent")
                    nc.vector.tensor_copy(dent, p2[0][64:128, :])
                    nc.gpsimd.dma_start(out=dbg_den_d[:, :], in_=dent)
                # stage both heads' denominators to SBUF before the custom-DVE
                # reciprocal: reciprocal_approx_fast reading PSUM at a shifted
                # base silently corrupts on hardware (sim-only correct)
                sums = inv_pool.tile([128, SQW], F32, tag="sums", name="sums")
                nc.vector.tensor_copy(sums[0:64, :], p2[0][64:128, :])
                nc.vector.tensor_copy(sums[64:128, :], p2[1][64:128, :])
                invb = inv_pool.tile([128, SQW], F32, tag="invb", name="invb")
                nc.vector.reciprocal_approx_fast(out=invb, in_=sums)
                for hi in (0, 1):
                    nc.vector.tensor_mul(
                        at[hi * 64:(hi + 1) * 64, :], p2[hi][0:64, :],
                        invb[hi * 64:(hi + 1) * 64, :],
                    )
                attnt[(m, j)] = at
                if DEBUG and (j, m) == (0, 0):
                    nc.gpsimd.dma_start(out=dbg_at_d[:, :], in_=at)

            def oproj_units(j, sc):
                """Generator: 4 units (one per 512-col block of Wo); each
                block's DMA starts as soon as its copy lands."""
                r0 = j * SQW + sc * 128
                for hc in range(4):
                    # the tail (last j) has no live attention pairs, so the
                    # pout2 banks are free — alternate to double-buffer deeper
                    pool = pout2_pool if (j == NJ - 1 and hc % 2) else pqkv_pool
                    po = pool.tile([128, 512], F32,
                                   tag="p2" if pool is pout2_pool else "qkv",
                                   name="po")
                    for mm in range(NPAIR):
                        nc.tensor.matmul(
                            po, attnt[(mm, j)][:, sc * 128:(sc + 1) * 128],
                            wot[:, mm, hc * 512:(hc + 1) * 512],
                            start=(mm == 0), stop=(mm == NPAIR - 1),
                        )
                    ot = ost_pool.tile([128, 512], BF16, tag="ot", name="ot")
                    if j == NJ - 1:
                        # tail: no exps remain, scalar is idle and the vector
                        # queue is backed up with the last pair's normalize
                        nc.scalar.copy(ot, po)
                    else:
                        nc.vector.tensor_copy(ot, po)
                    nc.gpsimd.dma_start(
                        out=out_d[r0:r0 + 128, hc * 512:(hc + 1) * 512], in_=ot
                    )
                    if hc == 3 and sc == 3:
                        for mm in range(NPAIR):
                            attnt.pop((mm, j))
                    yield

            def drive(pairs, fillers):
                """Run pair generators in order, spreading filler units
                evenly (Bresenham) across the total chunk count."""
                from itertools import chain
                n_chunks = sum(n for _, n in pairs)
                n_fill = sum(n for _, n in fillers)
                fit = chain(*(g for g, _ in fillers))
                done_c = emitted_f = 0
                for g, n in pairs:
                    for _ in g:
                        done_c += 1
                        target = (n_fill * done_c) // max(1, n_chunks)
                        while emitted_f < target:
                            next(fit, None)
                            emitted_f += 1
                for _ in fit:
                    pass

            # ---------------- schedule -----------------------------------
            QKV_ORDER = (4, 0, 5, 1, 2, 3)   # K first, V early, q pairs after

            # j=0 head: all of QKV(0) inline — its ~21us of PE work hides the
            # rope chains, so attention never waits on the vector engine
            for m in QKV_ORDER:
                for _ in qkv_units(0, m):
                    pass
            drive([(attention_pair_units(0, 0), 4)],
                  [(qkv_units(1, 4), 4), (qkv_units(1, 0), 4)])
            drive([(attention_pair_units(0, 1), 4)],
                  [(qkv_units(1, 5), 6), (qkv_units(1, 1), 4)])
            drive([(attention_pair_units(0, 2), 4)],
                  [(qkv_units(1, 2), 4)])
            drive([(attention_pair_units(0, 3), 4)],
                  [(qkv_units(1, 3), 4)])
            xt_tiles.pop(0)

            for j in range(1, NJ):
                if j + 2 < NJ:
                    load_xt(j + 2)
                nsk = 4 * (j + 1)
                pairs = [(attention_pair_units(j, m), nsk) for m in range(NPAIR)
                         if (j, m) not in _done_pairs]
                fillers = []
                if j + 1 < NJ:
                    fillers += [(qkv_units(j + 1, m), 6 if m == 5 else 4)
                                for m in QKV_ORDER]
                fillers += [(oproj_units(j - 1, sc), 4) for sc in range(4)]
                if j == NJ - 2:
                    # pull two of the last chunk's head-pairs forward so the
                    # tail window keeps the PE fed
                    pairs += [(attention_pair_units(NJ - 1, 0), 4 * NJ),
                              (attention_pair_units(NJ - 1, 1), 4 * NJ)]
                drive(pairs, fillers)
                xt_tiles.pop(j, None)
            for sc in range(4):
                for _ in oproj_units(NJ - 1, sc):
                    pass

    nc.finalize()
    _CACHED["nc"] = nc
    return nc


def _prep_inputs(cos, sin, hidden_states, per_head_scale, Wqkv, Wo):
    """Build the 8 per-core input maps (host-side, free)."""
    import ml_dtypes
    cos = np.asarray(cos, np.float32)
    sin = np.asarray(sin, np.float32)
    hs = np.asarray(hidden_states, np.float32)
    phs = np.asarray(per_head_scale, np.float32)
    Wqkv = np.asarray(Wqkv, np.float32)
    Wo = np.asarray(Wo, np.float32)

    cost = np.ascontiguousarray(np.vstack([cos.T, cos.T])).astype(ml_dtypes.bfloat16)
    st = sin.T.copy()
    st[0:32] *= -1.0
    # swap32-permuted: the sin factor for output row r is stored at row
    # partner(r), matching the partition of the swapped q operand
    stx = np.concatenate([st[32:64], st[0:32]], axis=0)
    sints = np.ascontiguousarray(np.vstack([stx, stx])).astype(ml_dtypes.bfloat16)

    tri01 = np.zeros((128, 128), np.float32)
    for p in range(128):
        tri01[p, p:] = 1.0
    ones_pad = np.zeros((128, 8), np.float32)
    ones_pad[:, 0] = 1.0
    id128 = np.eye(128, dtype=np.float32)
    constsb = np.ascontiguousarray(
        np.concatenate([tri01, ones_pad, id128], axis=1)
    ).astype(ml_dtypes.bfloat16)

    xt_b = [np.ascontiguousarray(hs[b].T).astype(ml_dtypes.bfloat16) for b in range(B)]

    in_maps = []
    for c in range(8):
        b, g = c // 4, c % 4
        hq0 = NH * g
        wq = Wqkv[hq0 * D:(hq0 + NH) * D, :].copy()
        for h in range(NH):
            wq[h * D:(h + 1) * D] *= (
                ROPE_MSCALE * SM_SCALE * phs[b, hq0 + h]
            )
        kv0 = H * D + NKV * g * D
        wk = Wqkv[kv0:kv0 + NKV * D, :] * ROPE_MSCALE
        v0 = (H + K) * D + NKV * g * D
        wv = Wqkv[v0:v0 + NKV * D, :]
        wqkv_c = np.ascontiguousarray(np.concatenate([wq, wk, wv], axis=0).T).astype(ml_dtypes.bfloat16)
        in_maps.append({
            "xt": xt_b[b],
            "wqkv": wqkv_c,
            "wo": np.ascontiguousarray(
                Wo[:, hq0 * D:(hq0 + NH) * D].T
            ).astype(ml_dtypes.bfloat16),
            "cost": cost,
            "sints": sints,
            "constsb": constsb,
        })
    return in_maps


def kernel(cos, sin, hidden_states, per_head_scale, Wqkv, Wo, _trace=False):
    nc = _build()
    in_maps = _prep_inputs(cos, sin, hidden_states, per_head_scale, Wqkv, Wo)
    res = run_bass_kernel_spmd(nc, in_maps, core_ids=list(range(8)), trace=_trace)
    _CACHED["last_results"] = res
    out = np.stack([
        sum(res.results[b * 4 + g]["out"].astype(np.float64) for g in range(4))
        for b in range(B)
    ]).astype(np.float32)
    return out
